# revision 4
# baseline (speedup 1.0000x reference)
"""CTRGC kernel for Trainium2 (Bass/Tile), 8-core SPMD, bf16, v2.

Sharding: core k = branch k//2 x batch half k%2 (16 of 32 samples).
Within a core, samples are processed in PAIRS: partition 64*s + c holds
channel c of pair-sample s. Per-sample weights are block-diagonalized on
host so one matmul serves both samples (w3bd/w12bd/w4bd).

Per (branch, sample) math (C=64, R=8, T=256, V=25):
  xm  = mean_t x; x1 = W1 xm; x2 = W2 xm                 [8,25]
  att[r,u,v] = tanh(x1[r,u]-x2[r,v] + (b1-b2)[r])        [8,25,25]
  a   = W4 att + b4 + A                                  [64,25,25]
  x3  = W3 x + b3                                        [64,256,25]
  out[c,t,u] = sum_v a[c,u,v] x3[c,t,v]                  [64,256,25]

The v-contraction of step 6 needs v on partitions for both operands.
Instead of a DRAM bounce, both transposes use the XBAR DMA-transpose
(dma_start(transpose=True)): for input [P, F] it maps flat free index
f to out[f%128, f//128, p]. With x3sb laid out [c, t, v32] (v padded to
32) this yields x3T[32*(t%4)+v, t//4, (s,c)] -- four v-bands q=t%4.
a is evacuated 4x-replicated as a_sb4[c, u, q, v32] so aTx[32q+v, u,
(s,c)] has every u in every band. Step 6 then runs 512 small matmuls
per pair: stationary x3T[32q+v, b, c] (K=25, M=64), moving aTx[32q+v,
u, c] (N=25), out psum[64qq+b, ci, u] with tile_position (32q, 64qq).

t-sum runs on PE as 16 PSUM-accumulated identity matmuls sharing xt.
loop_reps wraps everything in a hardware For_i loop (timing only).
"""

import numpy as np

try:
    import concourse  # noqa: F401
except ImportError:  # pragma: no cover
    import sys
    sys.path.insert(0, "/opt/trn_rl_repo")

_CACHE = {}


def _build_nc(loop_reps=1, skip_xbar=False, skip_s6=False, depth=2,
              fake_dep=False):
    from concourse import bacc, tile
    from concourse.bass import mybir

    f32 = mybir.dt.float32
    bf16 = mybir.dt.bfloat16
    ALU = mybir.AluOpType
    ACT = mybir.ActivationFunctionType
    AX = mybir.AxisListType

    nc = bacc.Bacc(None, target_bir_lowering=False)
    x_d = nc.declare_dram_parameter("x", [8, 128, 256, 25], bf16,
                                    isOutput=False)
    w3bd_d = nc.declare_dram_parameter("w3bd", [128, 128], bf16,
                                       isOutput=False)
    eye_d = nc.declare_dram_parameter("eye", [128, 128], bf16, isOutput=False)
    w12bd_d = nc.declare_dram_parameter("w12bd", [128, 2, 16], bf16,
                                        isOutput=False)
    w4bd_d = nc.declare_dram_parameter("w4bd", [16, 128], bf16,
                                       isOutput=False)
    b12_d = nc.declare_dram_parameter("b12", [16, 1], f32, isOutput=False)
    b3_d = nc.declare_dram_parameter("b3", [128, 1], f32, isOutput=False)
    apb4_d = nc.declare_dram_parameter("apb4", [128, 25, 32], bf16,
                                       isOutput=False)
    out_d = nc.declare_dram_parameter("out", [8, 128, 2, 2, 64, 25], bf16,
                                      isOutput=True)

    with tile.TileContext(nc) as tc:
        with (
            tc.tile_pool(name="const", bufs=1) as cpool,
            tc.tile_pool(name="xin", bufs=2) as xpool,
            tc.tile_pool(name="outp", bufs=2) as outpool,
            tc.tile_pool(name="small", bufs=2) as spool,
            tc.tile_pool(name="ps_ts", bufs=2, space="PSUM") as ps_ts,
            tc.tile_pool(name="ps_sm", bufs=2, space="PSUM") as ps_sm,
            tc.tile_pool(name="ps_x3", bufs=2, space="PSUM") as ps_x3,
            tc.tile_pool(name="ps_s6", bufs=2, space="PSUM") as ps_s6,
        ):
            w3bd = cpool.tile([128, 128], bf16)
            nc.sync.dma_start(w3bd[:], w3bd_d[:])
            eye = cpool.tile([128, 128], bf16)
            nc.sync.dma_start(eye[:], eye_d[:])
            w12bd = cpool.tile([128, 2, 16], bf16)
            nc.sync.dma_start(w12bd[:], w12bd_d[:])
            w4bd = cpool.tile([128, 128], bf16)
            nc.sync.dma_start(w4bd[0:16], w4bd_d[:])
            b12 = cpool.tile([128, 1], f32)
            nc.sync.dma_start(b12[0:16], b12_d[:])
            b3 = cpool.tile([128, 1], f32)
            nc.sync.dma_start(b3[:], b3_d[:])
            apb4 = cpool.tile([128, 25, 32], bf16)
            nc.sync.dma_start(apb4[:], apb4_d[:])

            # Persistent double-buffered tiles; v-pads (25:32) are zeroed
            # once and never rewritten, so the XBAR reads defined data.
            x3sbs = [cpool.tile([128, 256, 32], bf16, tag=f"x3sb{k}",
                                name=f"x3sb{k}") for k in range(2)]
            a4s = [cpool.tile([128, 25, 4, 32], bf16, tag=f"a4_{k}",
                              name=f"a4_{k}") for k in range(2)]
            x3Ts = [cpool.tile([128, 64, 128], bf16, tag=f"x3T{k}",
                               name=f"x3T{k}") for k in range(depth)]
            aTxs = [cpool.tile([128, 25, 128], bf16, tag=f"aTx{k}",
                               name=f"aTx{k}") for k in range(depth)]
            if fake_dep:
                fx3T = cpool.tile([128, 64, 128], bf16, name="fx3T")
                faTx = cpool.tile([128, 25, 128], bf16, name="faTx")
                nc.gpsimd.memset(fx3T[:], 0.0)
                nc.gpsimd.memset(faTx[:], 0.0)
            else:
                fx3T = faTx = None
            for k in range(2):
                nc.gpsimd.memset(x3sbs[k][:], 0.0)
                nc.gpsimd.memset(a4s[k][:], 0.0)
                if skip_xbar:
                    nc.gpsimd.memset(x3Ts[k][:], 0.0)
                    nc.gpsimd.memset(aTxs[k][:], 0.0)

            def pair_body(p):
                x3sb = x3sbs[p % 2]
                a_sb4 = a4s[p % 2]
                x3T = x3Ts[p % depth]
                aTx = aTxs[p % depth]

                xt = xpool.tile([128, 256, 25], bf16, tag="xt")
                nc.sync.dma_start(xt[:], x_d[p])

                # --- t-sum via accumulated identity matmuls ---
                ts_ps = ps_ts.tile([128, 16, 25], f32, tag="ts")
                for j in range(16):
                    nc.tensor.matmul(ts_ps[:], eye[:],
                                     xt[:, 16 * j:16 * j + 16, :],
                                     start=(j == 0), stop=(j == 15))
                xsum = spool.tile([128, 25], bf16, tag="xsum")
                with nc.allow_low_precision(
                        reason="16-partial f32 sum stored bf16 for matmul"):
                    nc.vector.tensor_reduce(
                        out=xsum[:],
                        in_=ts_ps[:].rearrange("p t v -> p v t"),
                        axis=AX.X, op=ALU.add)

                # --- x1/x2 (weights pre-scaled by 1/T on host) ---
                x12_ps = ps_sm.tile([128, 2, 25], f32, tag="sm")
                for w in range(2):
                    nc.tensor.matmul(x12_ps[0:16, w, :], w12bd[:, w, :],
                                     xsum[:], start=True, stop=True)
                x12_sb = spool.tile([128, 2, 25], f32, tag="x12sb")
                nc.vector.tensor_copy(x12_sb[0:16], x12_ps[0:16])

                # --- att[r,u,v] = tanh(x1[r,u] - x2[r,v] + (b1-b2)[r]) ---
                attp = spool.tile([128, 25, 25], bf16, tag="attp")
                x1b = x12_sb[0:16, 0:1, :].rearrange(
                    "r o u -> r u o").broadcast_to([16, 25, 25])
                x2b = x12_sb[0:16, 1:2, :].broadcast_to([16, 25, 25])
                nc.gpsimd.tensor_tensor(attp[0:16], x1b, x2b,
                                        op=ALU.subtract)
                att = spool.tile([128, 25, 25], bf16, tag="att")
                nc.scalar.activation(att[0:16], attp[0:16], ACT.Tanh,
                                     bias=b12[0:16], scale=1.0)
                att_f = att[0:16].rearrange("r u v -> r (u v)")

                # --- a = W4 att + (A + b4), evacuated 4x q-replicated ---
                apb4A = apb4[:, 0:20, 0:25]
                apb4B = apb4[:, 20:25, 0:25]
                aA = ps_sm.tile([128, 500], f32, tag="sm")
                nc.tensor.matmul(aA[:], w4bd[0:16], att_f[:, 0:500],
                                 start=True, stop=False)
                nc.tensor.matmul(aA[:].rearrange("p (u v) -> p u v", u=20),
                                 eye[:], apb4A, start=False, stop=True)
                aB = ps_sm.tile([128, 125], f32, tag="sm")
                nc.tensor.matmul(aB[:], w4bd[0:16], att_f[:, 500:625],
                                 start=True, stop=False)
                nc.tensor.matmul(aB[:].rearrange("p (u v) -> p u v", u=5),
                                 eye[:], apb4B, start=False, stop=True)
                nc.scalar.activation(
                    a_sb4[:, 0:20, :, 0:25],
                    aA[:].rearrange("p (u v) -> p u () v",
                                    u=20).broadcast_to([128, 20, 4, 25]),
                    ACT.Copy)
                nc.vector.tensor_copy(
                    a_sb4[:, 20:25, :, 0:25],
                    aB[:].rearrange("p (u v) -> p u () v",
                                    u=5).broadcast_to([128, 5, 4, 25]))

                # --- aTx[32q+v, u, (s,c)] via XBAR transpose ---
                if not skip_xbar:
                    nc.sync.dma_start(aTx[:], a_sb4[:], transpose=True)

                # --- x3 = W3 x + b3, laid out [c, t, v32] ---
                for m in range(16):
                    t0 = 16 * m
                    x3_ps = ps_x3.tile([128, 16, 25], f32, tag="x3")
                    nc.tensor.matmul(x3_ps[:], w3bd[:],
                                     xt[:, t0:t0 + 16, :],
                                     start=True, stop=True)
                    dst = x3sb[:, t0:t0 + 16, 0:25]
                    bb = b3[:].rearrange("p o -> p o ()").broadcast_to(
                        [128, 16, 25])
                    if m % 2 == 0:
                        nc.vector.tensor_tensor(dst, x3_ps[:], bb, op=ALU.add)
                    else:
                        nc.scalar.activation(dst, x3_ps[:], ACT.Identity,
                                             bias=b3[:], scale=1.0)

                # --- x3T[32*(t%4)+v, t//4, (s,c)] via XBAR transpose ---
                if not skip_xbar:
                    nc.sync.dma_start(x3T[:, 0:32, :], x3sb[:, 0:128, :],
                                      transpose=True)
                    nc.sync.dma_start(x3T[:, 32:64, :], x3sb[:, 128:256, :],
                                      transpose=True)

            def back_body(p):
                x3T = x3Ts[p % depth]
                aTx = aTxs[p % depth]
                if fake_dep:
                    x3T, aTx = x3Ts[p % depth], aTxs[p % depth]
                    x3T = w3bd[:].rearrange("p (a b) -> p a b", a=1) if False                         else x3T
                out_sb = outpool.tile([128, 2, 2, 64, 25], bf16, tag="out_sb")
                k = 0
                for s in range(2):
                    for cg in range(4):
                        for ab in range(2):
                            s6 = ps_s6.tile([128, 16, 25], f32, tag="s6")
                            nci = 1 if skip_s6 else 16
                            for ci in range(nci):
                                c = 64 * s + 16 * cg + ci
                                for qq in range(2):
                                    q = 2 * ab + qq
                                    lhs = (fx3T if fake_dep else
                                           x3T)[32 * q:32 * q + 25, :, c]
                                    rhs = (faTx if fake_dep else
                                           aTx)[32 * q:32 * q + 25, :, c]
                                    nc.tensor.matmul(
                                        s6[64 * qq:64 * qq + 64, ci, :],
                                        lhs, rhs,
                                        start=True, stop=True,
                                        tile_position=(32 * q, 64 * qq))
                            dst = out_sb[:, s, ab, 16 * cg:16 * cg + 16, :]
                            if k % 2 == 0:
                                nc.vector.tensor_copy(dst, s6[:])
                            else:
                                nc.scalar.activation(dst, s6[:], ACT.Copy)
                            k += 1
                nc.sync.dma_start(out_d[p], out_sb[:])

            if loop_reps > 1:
                _hints = (mybir.EngineType.PE, mybir.EngineType.DVE,
                          mybir.EngineType.Activation, mybir.EngineType.SP,
                          mybir.EngineType.Pool)
                with tc.For_i(0, loop_reps, 1, hint_engines=_hints):
                    for p in range(8):
                        pair_body(p)
                        if p > 0:
                            back_body(p - 1)
                    back_body(7)
            else:
                for p in range(8):
                    pair_body(p)
                    if p > 0:
                        back_body(p - 1)
                back_body(7)

    nc.compile()
    return nc


def _prep_core(x_half, A_b, W1, B1, W2, B2, W3, B3, W4, B4):
    f = np.float32
    import ml_dtypes
    bf16 = ml_dtypes.bfloat16

    xv = np.ascontiguousarray(x_half).reshape(8, 128, 256, 25)

    w3bd = np.zeros((128, 128), f)
    w12bd = np.zeros((128, 2, 16), f)
    w4bd = np.zeros((16, 128), f)
    b12 = np.zeros((16, 1), f)
    b3 = np.zeros((128, 1), f)
    apb4 = np.zeros((128, 25, 32), f)
    for s in (0, 1):
        o = 64 * s
        w3bd[o:o + 64, o:o + 64] = W3.T
        w12bd[o:o + 64, 0, 8 * s:8 * s + 8] = (W1 / 256.0).T
        w12bd[o:o + 64, 1, 8 * s:8 * s + 8] = (W2 / 256.0).T
        w4bd[8 * s:8 * s + 8, o:o + 64] = W4.T
        b12[8 * s:8 * s + 8, 0] = B1 - B2
        b3[o:o + 64, 0] = B3
        apb4[o:o + 64, :, 0:25] = A_b[None, :, :] + B4[:, None, None]
    eye = np.eye(128, dtype=f)

    return {
        "x": xv.astype(bf16),
        "w3bd": w3bd.astype(bf16),
        "eye": eye.astype(bf16),
        "w12bd": w12bd.astype(bf16),
        "w4bd": w4bd.astype(bf16),
        "b12": b12,
        "b3": b3,
        "apb4": apb4.astype(bf16),
    }


def _unshard_core(outbuf):
    """outbuf [8, 128, 2, 2, 64, 25] bf16 -> [16, 64, 256, 25] f32.

    out[pair, 64*qq+b, s, ab, c, u] holds sample 2*pair+s, channel c,
    t = 4*b + 2*ab + qq, u.
    """
    ob = np.asarray(outbuf, dtype=np.float32)
    ob = ob.reshape(8, 2, 64, 2, 2, 64, 25)       # [p, qq, b, s, ab, c, u]
    ob = ob.transpose(0, 3, 5, 2, 4, 1, 6)        # [p, s, c, b, ab, qq, u]
    ob = ob.reshape(16, 64, 256, 25)              # t = 4b + 2ab + qq
    return ob


def kernel(**inputs):
    from concourse.bass_utils import run_bass_kernel_spmd

    if "nc" not in _CACHE:
        _CACHE["nc"] = _build_nc()
    nc = _CACHE["nc"]

    A = np.asarray(inputs["A"], dtype=np.float32)
    xs = [np.asarray(inputs[k], dtype=np.float32)
          for k in ("jo", "bo", "jm", "bm")]
    W = {k: np.asarray(inputs[k], dtype=np.float32)
         for k in ("W1", "B1", "W2", "B2", "W3", "B3", "W4", "B4")}

    in_maps = []
    for k in range(8):
        b, h = k // 2, k % 2
        in_maps.append(_prep_core(
            xs[b][16 * h:16 * (h + 1)], A[b],
            W["W1"][b], W["B1"][b], W["W2"][b], W["B2"][b],
            W["W3"][b], W["B3"][b], W["W4"][b], W["B4"][b],
        ))

    res = run_bass_kernel_spmd(nc, in_maps, list(range(8))).results

    outs = []
    for b in range(4):
        parts = [_unshard_core(res[2 * b + h]["out"]) for h in range(2)]
        outs.append(np.concatenate(parts, axis=0))
    return tuple(outs)


# revision 5
# speedup vs baseline: 1.0888x; 1.0888x over previous
"""CTRGC kernel for Trainium2 (Bass/Tile), 8-core SPMD, bf16, v2.

Sharding: core k = branch k//2 x batch half k%2 (16 of 32 samples).
Within a core, samples are processed in PAIRS: partition 64*s + c holds
channel c of pair-sample s. Per-sample weights are block-diagonalized on
host so one matmul serves both samples (w3bd/w12bd/w4bd).

Per (branch, sample) math (C=64, R=8, T=256, V=25):
  xm  = mean_t x; x1 = W1 xm; x2 = W2 xm                 [8,25]
  att[r,u,v] = tanh(x1[r,u]-x2[r,v] + (b1-b2)[r])        [8,25,25]
  a   = W4 att + b4 + A                                  [64,25,25]
  x3  = W3 x + b3                                        [64,256,25]
  out[c,t,u] = sum_v a[c,u,v] x3[c,t,v]                  [64,256,25]

The v-contraction of step 6 needs v on partitions for both operands.
Instead of a DRAM bounce, both transposes use the XBAR DMA-transpose
(dma_start(transpose=True)): for input [P, F] it maps flat free index
f to out[f%128, f//128, p]. With x3sb laid out [c, t, v32] (v padded to
32) this yields x3T[32*(t%4)+v, t//4, (s,c)] -- four v-bands q=t%4.
a is evacuated 4x-replicated as a_sb4[c, u, q, v32] so aTx[32q+v, u,
(s,c)] has every u in every band. Step 6 then runs 512 small matmuls
per pair: stationary x3T[32q+v, b, c] (K=25, M=64), moving aTx[32q+v,
u, c] (N=25), out psum[64qq+b, ci, u] with tile_position (32q, 64qq).

t-sum runs on PE as 16 PSUM-accumulated identity matmuls sharing xt.
loop_reps wraps everything in a hardware For_i loop (timing only).
"""

import numpy as np

try:
    import concourse  # noqa: F401
except ImportError:  # pragma: no cover
    import sys
    sys.path.insert(0, "/opt/trn_rl_repo")

_CACHE = {}


def _build_nc(loop_reps=1, skip_xbar=False, skip_s6=False, depth=2,
              fake_dep=False):
    from concourse import bacc, tile
    from concourse.bass import mybir

    f32 = mybir.dt.float32
    bf16 = mybir.dt.bfloat16
    ALU = mybir.AluOpType
    ACT = mybir.ActivationFunctionType
    AX = mybir.AxisListType

    nc = bacc.Bacc(None, target_bir_lowering=False)
    x_d = nc.declare_dram_parameter("x", [8, 128, 256, 25], bf16,
                                    isOutput=False)
    w3bd_d = nc.declare_dram_parameter("w3bd", [128, 128], bf16,
                                       isOutput=False)
    eye_d = nc.declare_dram_parameter("eye", [128, 128], bf16, isOutput=False)
    w12bd_d = nc.declare_dram_parameter("w12bd", [128, 2, 16], bf16,
                                        isOutput=False)
    w4bd_d = nc.declare_dram_parameter("w4bd", [16, 128], bf16,
                                       isOutput=False)
    b12_d = nc.declare_dram_parameter("b12", [16, 1], f32, isOutput=False)
    b3_d = nc.declare_dram_parameter("b3", [128, 1], f32, isOutput=False)
    apb4_d = nc.declare_dram_parameter("apb4", [128, 25, 32], bf16,
                                       isOutput=False)
    out_d = nc.declare_dram_parameter("out", [8, 128, 2, 2, 64, 25], bf16,
                                      isOutput=True)

    with tile.TileContext(nc) as tc:
        with (
            tc.tile_pool(name="const", bufs=1) as cpool,
            tc.tile_pool(name="xin", bufs=2) as xpool,
            tc.tile_pool(name="outp", bufs=2) as outpool,
            tc.tile_pool(name="small", bufs=2) as spool,
            tc.tile_pool(name="ps_ts", bufs=1, space="PSUM") as ps_ts,
            tc.tile_pool(name="ps_sm", bufs=1, space="PSUM") as ps_sm,
            tc.tile_pool(name="ps_x3", bufs=3, space="PSUM") as ps_x3,
            tc.tile_pool(name="ps_s6", bufs=3, space="PSUM") as ps_s6,
        ):
            w3bd = cpool.tile([128, 128], bf16)
            nc.sync.dma_start(w3bd[:], w3bd_d[:])
            eye = cpool.tile([128, 128], bf16)
            nc.sync.dma_start(eye[:], eye_d[:])
            w12bd = cpool.tile([128, 2, 16], bf16)
            nc.sync.dma_start(w12bd[:], w12bd_d[:])
            w4bd = cpool.tile([128, 128], bf16)
            nc.sync.dma_start(w4bd[0:16], w4bd_d[:])
            b12 = cpool.tile([128, 1], f32)
            nc.sync.dma_start(b12[0:16], b12_d[:])
            b3 = cpool.tile([128, 1], f32)
            nc.sync.dma_start(b3[:], b3_d[:])
            apb4 = cpool.tile([128, 25, 32], bf16)
            nc.sync.dma_start(apb4[:], apb4_d[:])

            # Persistent double-buffered tiles; v-pads (25:32) are zeroed
            # once and never rewritten, so the XBAR reads defined data.
            x3sbs = [cpool.tile([128, 256, 32], bf16, tag=f"x3sb{k}",
                                name=f"x3sb{k}") for k in range(2)]
            a4s = [cpool.tile([128, 25, 4, 32], bf16, tag=f"a4_{k}",
                              name=f"a4_{k}") for k in range(2)]
            x3Ts = [cpool.tile([128, 64, 128], bf16, tag=f"x3T{k}",
                               name=f"x3T{k}") for k in range(depth)]
            aTxs = [cpool.tile([128, 25, 128], bf16, tag=f"aTx{k}",
                               name=f"aTx{k}") for k in range(depth)]
            if fake_dep:
                fx3T = cpool.tile([128, 64, 128], bf16, name="fx3T")
                faTx = cpool.tile([128, 25, 128], bf16, name="faTx")
                nc.gpsimd.memset(fx3T[:], 0.0)
                nc.gpsimd.memset(faTx[:], 0.0)
            else:
                fx3T = faTx = None
            for k in range(2):
                nc.gpsimd.memset(x3sbs[k][:], 0.0)
                nc.gpsimd.memset(a4s[k][:], 0.0)
                if skip_xbar:
                    nc.gpsimd.memset(x3Ts[k][:], 0.0)
                    nc.gpsimd.memset(aTxs[k][:], 0.0)

            def pair_body(p):
                x3sb = x3sbs[p % 2]
                a_sb4 = a4s[p % 2]
                x3T = x3Ts[p % depth]
                aTx = aTxs[p % depth]

                xt = xpool.tile([128, 256, 25], bf16, tag="xt")
                nc.sync.dma_start(xt[:], x_d[p])

                # --- t-sum via accumulated identity matmuls ---
                ts_ps = ps_ts.tile([128, 16, 25], f32, tag="ts")
                for j in range(16):
                    nc.tensor.matmul(ts_ps[:], eye[:],
                                     xt[:, 16 * j:16 * j + 16, :],
                                     start=(j == 0), stop=(j == 15))
                xsum = spool.tile([128, 25], bf16, tag="xsum")
                with nc.allow_low_precision(
                        reason="16-partial f32 sum stored bf16 for matmul"):
                    nc.vector.tensor_reduce(
                        out=xsum[:],
                        in_=ts_ps[:].rearrange("p t v -> p v t"),
                        axis=AX.X, op=ALU.add)

                # --- x1/x2 (weights pre-scaled by 1/T on host) ---
                x12_ps = ps_sm.tile([128, 2, 25], f32, tag="sm")
                for w in range(2):
                    nc.tensor.matmul(x12_ps[0:16, w, :], w12bd[:, w, :],
                                     xsum[:], start=True, stop=True)
                x12_sb = spool.tile([128, 2, 25], f32, tag="x12sb")
                nc.vector.tensor_copy(x12_sb[0:16], x12_ps[0:16])

                # --- att[r,u,v] = tanh(x1[r,u] - x2[r,v] + (b1-b2)[r]) ---
                attp = spool.tile([128, 25, 25], bf16, tag="attp")
                x1b = x12_sb[0:16, 0:1, :].rearrange(
                    "r o u -> r u o").broadcast_to([16, 25, 25])
                x2b = x12_sb[0:16, 1:2, :].broadcast_to([16, 25, 25])
                nc.gpsimd.tensor_tensor(attp[0:16], x1b, x2b,
                                        op=ALU.subtract)
                att = spool.tile([128, 25, 25], bf16, tag="att")
                nc.scalar.activation(att[0:16], attp[0:16], ACT.Tanh,
                                     bias=b12[0:16], scale=1.0)
                att_f = att[0:16].rearrange("r u v -> r (u v)")

                # --- a = W4 att + (A + b4), evacuated 4x q-replicated ---
                apb4A = apb4[:, 0:20, 0:25]
                apb4B = apb4[:, 20:25, 0:25]
                aA = ps_sm.tile([128, 500], f32, tag="sm")
                nc.tensor.matmul(aA[:], w4bd[0:16], att_f[:, 0:500],
                                 start=True, stop=False)
                nc.tensor.matmul(aA[:].rearrange("p (u v) -> p u v", u=20),
                                 eye[:], apb4A, start=False, stop=True)
                aB = ps_sm.tile([128, 125], f32, tag="sm")
                nc.tensor.matmul(aB[:], w4bd[0:16], att_f[:, 500:625],
                                 start=True, stop=False)
                nc.tensor.matmul(aB[:].rearrange("p (u v) -> p u v", u=5),
                                 eye[:], apb4B, start=False, stop=True)
                nc.scalar.activation(
                    a_sb4[:, 0:20, :, 0:25],
                    aA[:].rearrange("p (u v) -> p u () v",
                                    u=20).broadcast_to([128, 20, 4, 25]),
                    ACT.Copy)
                nc.vector.tensor_copy(
                    a_sb4[:, 20:25, :, 0:25],
                    aB[:].rearrange("p (u v) -> p u () v",
                                    u=5).broadcast_to([128, 5, 4, 25]))

                # --- aTx[32q+v, u, (s,c)] via XBAR transpose ---
                if not skip_xbar:
                    nc.sync.dma_start(aTx[:], a_sb4[:], transpose=True)

                # --- x3 = W3 x + b3, laid out [c, t, v32] ---
                for m in range(16):
                    t0 = 16 * m
                    x3_ps = ps_x3.tile([128, 16, 25], f32, tag="x3")
                    nc.tensor.matmul(x3_ps[:], w3bd[:],
                                     xt[:, t0:t0 + 16, :],
                                     start=True, stop=True)
                    dst = x3sb[:, t0:t0 + 16, 0:25]
                    bb = b3[:].rearrange("p o -> p o ()").broadcast_to(
                        [128, 16, 25])
                    if m % 2 == 0:
                        nc.vector.tensor_tensor(dst, x3_ps[:], bb, op=ALU.add)
                    else:
                        nc.scalar.activation(dst, x3_ps[:], ACT.Identity,
                                             bias=b3[:], scale=1.0)

                # --- x3T[32*(t%4)+v, t//4, (s,c)] via XBAR transpose ---
                if not skip_xbar:
                    nc.sync.dma_start(x3T[:, 0:32, :], x3sb[:, 0:128, :],
                                      transpose=True)
                    nc.sync.dma_start(x3T[:, 32:64, :], x3sb[:, 128:256, :],
                                      transpose=True)

            def back_body(p):
                x3T = x3Ts[p % depth]
                aTx = aTxs[p % depth]
                if fake_dep:
                    x3T, aTx = x3Ts[p % depth], aTxs[p % depth]
                    x3T = w3bd[:].rearrange("p (a b) -> p a b", a=1) if False                         else x3T
                out_sb = outpool.tile([128, 2, 2, 64, 25], bf16, tag="out_sb")
                k = 0
                for s in range(2):
                    for cg in range(4):
                        for ab in range(2):
                            s6 = ps_s6.tile([128, 16, 25], f32, tag="s6")
                            nci = 1 if skip_s6 else 16
                            for ci in range(nci):
                                c = 64 * s + 16 * cg + ci
                                for qq in range(2):
                                    q = 2 * ab + qq
                                    lhs = (fx3T if fake_dep else
                                           x3T)[32 * q:32 * q + 25, :, c]
                                    rhs = (faTx if fake_dep else
                                           aTx)[32 * q:32 * q + 25, :, c]
                                    nc.tensor.matmul(
                                        s6[64 * qq:64 * qq + 64, ci, :],
                                        lhs, rhs,
                                        start=True, stop=True,
                                        tile_position=(32 * q, 64 * qq))
                            dst = out_sb[:, s, ab, 16 * cg:16 * cg + 16, :]
                            if k % 2 == 0:
                                nc.vector.tensor_copy(dst, s6[:])
                            else:
                                nc.scalar.activation(dst, s6[:], ACT.Copy)
                            k += 1
                nc.sync.dma_start(out_d[p], out_sb[:])

            if loop_reps > 1:
                _hints = (mybir.EngineType.PE, mybir.EngineType.DVE,
                          mybir.EngineType.Activation, mybir.EngineType.SP,
                          mybir.EngineType.Pool)
                with tc.For_i(0, loop_reps, 1, hint_engines=_hints):
                    for p in range(8):
                        pair_body(p)
                        if p > 0:
                            back_body(p - 1)
                    back_body(7)
            else:
                for p in range(8):
                    pair_body(p)
                    if p > 0:
                        back_body(p - 1)
                back_body(7)

    nc.compile()
    return nc


def _prep_core(x_half, A_b, W1, B1, W2, B2, W3, B3, W4, B4):
    f = np.float32
    import ml_dtypes
    bf16 = ml_dtypes.bfloat16

    xv = np.ascontiguousarray(x_half).reshape(8, 128, 256, 25)

    w3bd = np.zeros((128, 128), f)
    w12bd = np.zeros((128, 2, 16), f)
    w4bd = np.zeros((16, 128), f)
    b12 = np.zeros((16, 1), f)
    b3 = np.zeros((128, 1), f)
    apb4 = np.zeros((128, 25, 32), f)
    for s in (0, 1):
        o = 64 * s
        w3bd[o:o + 64, o:o + 64] = W3.T
        w12bd[o:o + 64, 0, 8 * s:8 * s + 8] = (W1 / 256.0).T
        w12bd[o:o + 64, 1, 8 * s:8 * s + 8] = (W2 / 256.0).T
        w4bd[8 * s:8 * s + 8, o:o + 64] = W4.T
        b12[8 * s:8 * s + 8, 0] = B1 - B2
        b3[o:o + 64, 0] = B3
        apb4[o:o + 64, :, 0:25] = A_b[None, :, :] + B4[:, None, None]
    eye = np.eye(128, dtype=f)

    return {
        "x": xv.astype(bf16),
        "w3bd": w3bd.astype(bf16),
        "eye": eye.astype(bf16),
        "w12bd": w12bd.astype(bf16),
        "w4bd": w4bd.astype(bf16),
        "b12": b12,
        "b3": b3,
        "apb4": apb4.astype(bf16),
    }


def _unshard_core(outbuf):
    """outbuf [8, 128, 2, 2, 64, 25] bf16 -> [16, 64, 256, 25] f32.

    out[pair, 64*qq+b, s, ab, c, u] holds sample 2*pair+s, channel c,
    t = 4*b + 2*ab + qq, u.
    """
    ob = np.asarray(outbuf, dtype=np.float32)
    ob = ob.reshape(8, 2, 64, 2, 2, 64, 25)       # [p, qq, b, s, ab, c, u]
    ob = ob.transpose(0, 3, 5, 2, 4, 1, 6)        # [p, s, c, b, ab, qq, u]
    ob = ob.reshape(16, 64, 256, 25)              # t = 4b + 2ab + qq
    return ob


def kernel(**inputs):
    from concourse.bass_utils import run_bass_kernel_spmd

    if "nc" not in _CACHE:
        _CACHE["nc"] = _build_nc()
    nc = _CACHE["nc"]

    A = np.asarray(inputs["A"], dtype=np.float32)
    xs = [np.asarray(inputs[k], dtype=np.float32)
          for k in ("jo", "bo", "jm", "bm")]
    W = {k: np.asarray(inputs[k], dtype=np.float32)
         for k in ("W1", "B1", "W2", "B2", "W3", "B3", "W4", "B4")}

    in_maps = []
    for k in range(8):
        b, h = k // 2, k % 2
        in_maps.append(_prep_core(
            xs[b][16 * h:16 * (h + 1)], A[b],
            W["W1"][b], W["B1"][b], W["W2"][b], W["B2"][b],
            W["W3"][b], W["B3"][b], W["W4"][b], W["B4"][b],
        ))

    res = run_bass_kernel_spmd(nc, in_maps, list(range(8))).results

    outs = []
    for b in range(4):
        parts = [_unshard_core(res[2 * b + h]["out"]) for h in range(2)]
        outs.append(np.concatenate(parts, axis=0))
    return tuple(outs)
